# revision 25
# baseline (speedup 1.0000x reference)
"""Encoder self-attention (RMSNorm + fused QKV + qk-norm + SDPA + scaled o_proj
+ residual) on 8 NeuronCores, data-parallel over the batch dim N=8.

v2: head-pair-major pipeline. QKV for one head-pair (q|k|v column blocks via a
step-sliced rhs) -> qk-norm (scale-invariant, so input-RMSNorm is skipped for
q/k and only applied to v) -> DMA-transpose into [chan, tok] layout -> scores
-> exp -> AV (with a ones column in v producing softmax denominators) ->
normalize via a K=1 outer-product broadcast matmul. All norm factors are
computed as exp(-0.5*ln(x)) so the ACT engine only ever needs one activation
table (ln/exp/square). Eviction/elementwise work uses any-engine ops so the
tile scheduler load-balances DVE/Pool/ACT.
"""

import numpy as np
import ml_dtypes
from contextlib import ExitStack

import concourse.bass as bass
import concourse.mybir as mybir
import concourse.tile as tile
from concourse import bacc
from concourse.bass import ts

F32 = mybir.dt.float32
F32R = mybir.dt.float32r
BF16 = mybir.dt.bfloat16
AF = mybir.ActivationFunctionType

P = 128
D = 768
L = 1024
NH = 12
HD = 64
TQ = L // P      # 8 token tiles
KC = D // P      # 6 contraction chunks
NHP = NH // 2    # 6 head pairs
EPS = 1e-6
LN8 = float(np.log(8.0))


def r(ap):
    return ap.bitcast(F32R)


def build_bass():
    nc = bacc.Bacc(None, target_bir_lowering=False)

    x_d = nc.dram_tensor("x", [L, D], F32, kind="ExternalInput")
    xT_d = nc.dram_tensor("xT", [D, L], BF16, kind="ExternalInput")
    wT_d = nc.dram_tensor("wT", [D, 3 * D], BF16, kind="ExternalInput")
    oT_d = nc.dram_tensor("oT", [D, D], BF16, kind="ExternalInput")
    ones_d = nc.dram_tensor("ones", [1, HD], F32R, kind="ExternalInput")
    out_d = nc.dram_tensor("out", [L, D], F32, kind="ExternalOutput")

    with tile.TileContext(nc) as tc, ExitStack() as ctx:
        persist = ctx.enter_context(tc.tile_pool(name="persist", bufs=1))
        ets_pool = ctx.enter_context(tc.tile_pool(name="ets", bufs=2))
        scr = ctx.enter_context(tc.tile_pool(name="scr", bufs=2))
        qkvp = ctx.enter_context(tc.tile_pool(name="qkvp", bufs=2, space="PSUM"))
        scp = ctx.enter_context(tc.tile_pool(name="scp", bufs=2, space="PSUM"))
        avp = ctx.enter_context(tc.tile_pool(name="avp", bufs=2, space="PSUM"))

        # ---- persistent SBUF ----
        xT_all = persist.tile([P, KC, L], BF16, tag="xT_all", name="xT_all")
        wT_all = persist.tile([P, KC, 18, P], BF16, tag="wT_all", name="wT_all")
        qnT = persist.tile([P, NHP, L], BF16, tag="qnT", name="qnT")
        knT = persist.tile([P, NHP, L], BF16, tag="knT", name="knT")
        v_sb = [
            persist.tile([P, NH, HD + 1], BF16, tag=f"v{j}", name=f"v{j}")
            for j in range(TQ)
        ]
        attnT = persist.tile([P, KC, L], BF16, tag="attnT", name="attnT")
        oT_all = persist.tile([P, KC, D], BF16, tag="oT_all", name="oT_all")
        rstd = persist.tile([P, TQ], F32, tag="rstd")
        ones1 = persist.tile([1, HD], F32R, tag="ones1")
        nc.sync.dma_start(out=ones1[:], in_=ones_d[:])
        eps_t = persist.tile([P, 1], F32, tag="eps_t")
        nc.any.memset(eps_t[:], EPS)
        eps64_t = persist.tile([P, 1], F32, tag="eps64_t")
        nc.any.memset(eps64_t[:], HD * EPS)
        ln8_t = persist.tile([P, 1], F32, tag="ln8_t")
        nc.any.memset(ln8_t[:], LN8)

        # ---- input DMAs ----
        for d in range(KC):
            nc.sync.dma_start(out=xT_all[:, d, :], in_=xT_d[ts(d, P), :])
            nc.sync.dma_start(
                out=wT_all[:, d, :, :],
                in_=wT_d[ts(d, P), :].rearrange("p (g c) -> p g c", c=P),
            )
        nc.sync.dma_start(
            out=oT_all[:], in_=oT_d[:].rearrange("(c p) e -> p c e", p=P)
        )
        for i in range(TQ):
            nc.any.memset(v_sb[i][:, :, HD : HD + 1], 1.0)

        # ---- rstd for all token tiles (only v needs it; qk-norm is
        # scale-invariant so the input RMSNorm cancels for q/k) ----
        for i in range(TQ):
            xs = scr.tile([P, D], F32, tag="xs")
            nc.sync.dma_start(out=xs[:], in_=x_d[ts(i, P), :])
            sqs = scr.tile([P, D], F32, tag="sqs", bufs=1)
            ssq = scr.tile([P, 1], F32, tag="ssq")
            nc.scalar.activation(sqs[:], xs[:], AF.Square, accum_out=ssq[:])
            lnv = scr.tile([P, 1], F32, tag="lnv")
            nc.scalar.activation(lnv[:], ssq[:], AF.Ln, scale=1.0 / D, bias=eps_t[:])
            nc.scalar.activation(
                rstd[:, i : i + 1], lnv[:], AF.Exp, scale=-0.5
            )

        # ---- main loop: head-pair major, QKV + attention pipelined ----
        for hp in range(NHP):
            # --- QKV + qk-norm + transposes for this head pair ---
            for i in range(TQ):
                ps = qkvp.tile([P, 3, P], F32, tag="qkv")
                for d in range(KC):
                    nc.tensor.matmul(
                        ps[:],
                        xT_all[:, d, ts(i, P)],
                        wT_all[:, d, hp::NHP, :],
                        start=(d == 0),
                        stop=(d == KC - 1),
                    )
                ps_qk = ps[:, 0:2, :].rearrange("p a (h d) -> p (a h) d", d=HD)
                # per-head sum of squares -> norm factors
                sqg = scr.tile([P, 4, HD], F32, tag="sqg")
                nc.scalar.activation(
                    sqg[:].rearrange("p h d -> p (h d)"),
                    ps_qk.rearrange("p h d -> p (h d)"),
                    AF.Square,
                )
                ssg = scr.tile([P, 4], F32, tag="ssg")
                nc.vector.tensor_reduce(
                    ssg[:], sqg[:], axis=mybir.AxisListType.X, op=mybir.AluOpType.add
                )
                l4 = scr.tile([P, 4], F32, tag="l4")
                nc.scalar.activation(l4[:], ssg[:], AF.Ln, bias=eps64_t[:])
                r4 = scr.tile([P, 4, 1], F32, tag="r4")
                nc.scalar.activation(r4[:, 0:2, 0], l4[:, 0:2], AF.Exp, scale=-0.5)
                # k-norm lacks the folded 1/sqrt(hd) attention scale -> *8
                nc.scalar.activation(
                    r4[:, 2:4, 0], l4[:, 2:4], AF.Exp, scale=-0.5, bias=ln8_t[:]
                )
                # normalize q/k to bf16, scale v by rstd
                tn = scr.tile([P, 4, HD], BF16, tag="tn", bufs=3)
                nc.any.tensor_mul(
                    tn[:], ps_qk, r4[:].to_broadcast((P, 4, HD))
                )
                nc.any.tensor_scalar_mul(
                    v_sb[i][:, 2 * hp : 2 * hp + 2, 0:HD],
                    ps[:, 2, :].rearrange("p (h d) -> p h d", d=HD),
                    rstd[:, i : i + 1],
                )
                tn2 = tn[:].rearrange("p h d -> p (h d)")
                nc.sync.dma_start_transpose(
                    out=qnT[:, hp, ts(i, P)], in_=tn2[:, 0:P]
                )
                nc.sync.dma_start_transpose(
                    out=knT[:, hp, ts(i, P)], in_=tn2[:, P : 2 * P]
                )

            # --- scores + exp ---
            ets = {}
            for jt in range(TQ):
                for hh in range(2):
                    off = HD * hh
                    sc = scp.tile([P, L], F32, tag="sc")
                    for ic in range(2):
                        nc.tensor.matmul(
                            sc[:, ts(ic, 512)],
                            knT[off : off + HD, hp, ts(jt, P)],
                            qnT[off : off + HD, hp, ts(ic, 512)],
                            start=True,
                            stop=True,
                            tile_position=(off, 0),
                        )
                    et = ets_pool.tile([P, L], BF16, tag=f"ets{jt}_{hh}")
                    nc.scalar.activation(et[:], sc[:], AF.Exp)
                    ets[(jt, hh)] = et

            # --- AV (ones column of v gives softmax denominators) ---
            rs = scr.tile([1, 2, L], F32, tag="rs", bufs=1)
            for hh in range(2):
                h = 2 * hp + hh
                for ic in range(2):
                    av = avp.tile([HD + 1, 512], F32, tag="av")
                    for jt in range(TQ):
                        nc.tensor.matmul(
                            av[:],
                            v_sb[jt][:, h, :],
                            ets[(jt, hh)][:, ts(ic, 512)],
                            start=(jt == 0),
                            stop=(jt == TQ - 1),
                        )
                    nc.any.tensor_copy(
                        attnT[HD * hh : HD * hh + HD, hp, ts(ic, 512)], av[0:HD, :]
                    )
                    nc.any.tensor_copy(
                        rs[0:1, hh, ts(ic, 512)], av[HD : HD + 1, :]
                    )
            rinv = scr.tile([1, 2, L], F32R, tag="rinv", bufs=1)
            with nc.allow_low_precision(reason="softmax denominators in fp32r"):
                nc.vector.reciprocal(rinv[:], rs[:])
            # --- normalize: broadcast 1/rowsum over 64 channels via K=1 matmul ---
            for hh in range(2):
                off = HD * hh
                for ic in range(2):
                    bc = avp.tile([HD + 1, 512], F32, tag="av")
                    nc.tensor.matmul(
                        bc[0:HD, :],
                        ones1[:],
                        rinv[0:1, hh, ts(ic, 512)],
                        start=True,
                        stop=True,
                    )
                    nc.any.tensor_mul(
                        attnT[off : off + HD, hp, ts(ic, 512)],
                        attnT[off : off + HD, hp, ts(ic, 512)],
                        bc[0:HD, :],
                    )

        # ---- o_proj + residual ----
        for i in range(TQ):
            o = scp.tile([P, L], F32, tag="sc")
            for c in range(KC):
                nc.tensor.matmul(
                    o[:, 0:512],
                    attnT[:, c, ts(i, P)],
                    oT_all[:, c, 0:512],
                    start=(c == 0),
                    stop=(c == KC - 1),
                )
            for c in range(KC):
                nc.tensor.matmul(
                    o[:, 512:D],
                    attnT[:, c, ts(i, P)],
                    oT_all[:, c, 512:D],
                    start=(c == 0),
                    stop=(c == KC - 1),
                )
            xs = scr.tile([P, D], F32, tag="xs")
            nc.sync.dma_start(out=xs[:], in_=x_d[ts(i, P), :])
            out_sb = scr.tile([P, D], F32, tag="osb")
            nc.any.tensor_add(out_sb[:, 0:512], o[:, 0:512], xs[:, 0:512])
            nc.any.tensor_add(out_sb[:, 512:D], o[:, 512:D], xs[:, 512:D])
            nc.sync.dma_start(out=out_d[ts(i, P), :], in_=out_sb[:])

    nc.compile()
    return nc


_NC = None


def _get_nc():
    global _NC
    if _NC is None:
        _NC = build_bass()
    return _NC


def make_in_maps(input_NHWD, qkv_weight, o_weight, o_scale):
    N = input_NHWD.shape[0]
    wT = np.ascontiguousarray(
        qkv_weight.reshape(3 * D, D).T.astype(np.float32)
    ).astype(ml_dtypes.bfloat16)
    oT = np.ascontiguousarray(
        (o_weight * o_scale[:, None]).T.astype(np.float32)
    ).astype(ml_dtypes.bfloat16)
    in_maps = []
    for i in range(N):
        xi = np.ascontiguousarray(input_NHWD[i].reshape(L, D).astype(np.float32))
        in_maps.append(
            {
                "x": xi,
                "xT": np.ascontiguousarray(xi.T).astype(ml_dtypes.bfloat16),
                "wT": wT,
                "oT": oT,
                "ones": np.ones((1, HD), dtype=np.float32),
            }
        )
    return in_maps


def kernel(input_NHWD, qkv_weight, o_weight, o_scale):
    import time
    from concourse.bass_utils import run_bass_kernel_spmd

    input_NHWD = np.asarray(input_NHWD)
    N, H, W, _ = input_NHWD.shape
    nc = _get_nc()
    in_maps = make_in_maps(
        np.asarray(input_NHWD),
        np.asarray(qkv_weight),
        np.asarray(o_weight),
        np.asarray(o_scale),
    )
    last_err = None
    for attempt in range(3):
        try:
            res = run_bass_kernel_spmd(nc, in_maps, list(range(N)))
            out = np.stack([res.results[i]["out"] for i in range(N)], axis=0)
            return out.reshape(N, H, W, D).astype(np.float32)
        except Exception as e:  # transient device wedge: clear + retry
            last_err = e
            try:
                import jax

                jax.clear_caches()
                jax.clear_backends()
            except Exception:
                pass
            time.sleep(10)
    raise last_err


# revision 27
# speedup vs baseline: 1.4494x; 1.4494x over previous
"""Encoder self-attention (RMSNorm + fused QKV + qk-norm + SDPA + scaled o_proj
+ residual) on 8 NeuronCores, data-parallel over the batch dim N=8.

v3: head-pair-major software pipeline.
- QKV for one head pair via a step-sliced rhs (q|k|v 128-col blocks).
- qk-norm is scale-invariant => input-RMSNorm applied only to v.
- q/k PSUM evicted to bf16 staging; per-head sum-of-squares on Pool/DVE;
  norm factors batched per head pair as exp(-0.5*ln(ssq+hd*eps)) on ACT
  (one pinned activation table: ln/exp/square -> no table thrash).
- Transposed to [chan, tok] via one combined DMA-transpose per token tile.
- scores -> exp(bf16) -> AV with a ones column producing softmax
  denominators -> reciprocal_approx_fast -> bf16 K=1 outer-product
  broadcast matmul -> normalize.
- Emission is software-pipelined: QKV(hp+1) is emitted before attention(hp)
  so the PE never waits for fresh transposes.
"""

import numpy as np
import ml_dtypes
from contextlib import ExitStack

import concourse.bass as bass
import concourse.mybir as mybir
import concourse.tile as tile
from concourse import bacc
from concourse.bass import ts

F32 = mybir.dt.float32
BF16 = mybir.dt.bfloat16
AF = mybir.ActivationFunctionType

P = 128
D = 768
L = 1024
NH = 12
HD = 64
TQ = L // P      # 8 token tiles
KC = D // P      # 6 contraction chunks
NHP = NH // 2    # 6 head pairs
EPS = 1e-6
# act_info.json act_func_sets index of natural_log_exp_and_others
ACT_SET_LN_EXP = 6


def build_bass():
    nc = bacc.Bacc(None, target_bir_lowering=False)

    x_d = nc.dram_tensor("x", [L, D], F32, kind="ExternalInput")
    xT_d = nc.dram_tensor("xT", [D, L], BF16, kind="ExternalInput")
    wT_d = nc.dram_tensor("wT", [D, 3 * D], BF16, kind="ExternalInput")
    oT_d = nc.dram_tensor("oT", [D, D], BF16, kind="ExternalInput")
    out_d = nc.dram_tensor("out", [L, D], F32, kind="ExternalOutput")

    with tile.TileContext(nc) as tc, ExitStack() as ctx:
        persist = ctx.enter_context(tc.tile_pool(name="persist", bufs=1))
        ets_pool = ctx.enter_context(tc.tile_pool(name="ets", bufs=2))
        scr = ctx.enter_context(tc.tile_pool(name="scr", bufs=2))
        qkvp = ctx.enter_context(tc.tile_pool(name="qkvp", bufs=2, space="PSUM"))
        scp = ctx.enter_context(tc.tile_pool(name="scp", bufs=2, space="PSUM"))
        avp = ctx.enter_context(tc.tile_pool(name="avp", bufs=2, space="PSUM"))

        # pin the ln/exp/square activation table once: every ACT func used
        # here is served by it, so no per-activation table reloads
        atl = mybir.InstLoadActFuncSet(
            name=nc.get_next_instruction_name(),
            ins=[],
            outs=[],
            act_func_set_id=ACT_SET_LN_EXP,
        )
        nc.scalar.add_instruction(atl)

        # ---- persistent SBUF ----
        xT_all = persist.tile([P, KC, L], BF16, tag="xT_all", name="xT_all")
        wT_all = persist.tile([P, KC, 18, P], BF16, tag="wT_all", name="wT_all")
        # qkT[:, 0] = qnT, qkT[:, 1] = knT  (chan-major, [chan, tok] blocks)
        qkT = persist.tile([P, 2, NHP, L], BF16, tag="qkT", name="qkT")
        v_sb = [
            persist.tile([P, NH, HD + 1], BF16, tag=f"v{j}", name=f"v{j}")
            for j in range(TQ)
        ]
        attnT = persist.tile([P, KC, L], BF16, tag="attnT", name="attnT")
        oT_all = persist.tile([P, KC, D], BF16, tag="oT_all", name="oT_all")
        rstd = persist.tile([P, TQ], F32, tag="rstd")
        ones_b = persist.tile([1, HD], BF16, tag="ones_b")
        nc.gpsimd.memset(ones_b[:], 1.0)
        eps_t = persist.tile([P, 1], F32, tag="eps_t")
        nc.gpsimd.memset(eps_t[:], EPS)
        eps64_t = persist.tile([P, 1], F32, tag="eps64_t")
        nc.gpsimd.memset(eps64_t[:], HD * EPS)

        # ---- input DMAs ----
        for d in range(KC):
            nc.sync.dma_start(out=xT_all[:, d, :], in_=xT_d[ts(d, P), :])
            nc.sync.dma_start(
                out=wT_all[:, d, :, :],
                in_=wT_d[ts(d, P), :].rearrange("p (g c) -> p g c", c=P),
            )
        nc.sync.dma_start(
            out=oT_all[:], in_=oT_d[:].rearrange("(c p) e -> p c e", p=P)
        )
        for i in range(TQ):
            nc.gpsimd.memset(v_sb[i][:, :, HD : HD + 1], 1.0)

        # ---- rstd for all token tiles (batched ln/exp) ----
        ssq_all = scr.tile([P, TQ], F32, tag="ssq_all", bufs=1)
        for pair in range(TQ // 2):
            xs = scr.tile([P, 2, D], F32, tag="xs", bufs=1)
            nc.sync.dma_start(
                out=xs[:],
                in_=x_d[ts(pair, 2 * P), :].rearrange("(a p) d -> p a d", p=P),
            )
            sqs = scr.tile([P, D], F32, tag="sqs", bufs=1)
            for j in range(2):
                i = 2 * pair + j
                nc.scalar.activation(
                    sqs[:], xs[:, j, :], AF.Square,
                    accum_out=ssq_all[:, i : i + 1],
                )
        lnr = scr.tile([P, TQ], F32, tag="lnr", bufs=1)
        nc.scalar.activation(lnr[:], ssq_all[:], AF.Ln, scale=1.0 / D, bias=eps_t[:])
        nc.scalar.activation(rstd[:], lnr[:], AF.Exp, scale=-0.5)

        # ================= software-pipelined main loop =================
        def emit_qkv(hp):
            """QKV + staging + sum-of-squares + batched norm + transposes."""
            tus = []
            ssg_all = scr.tile([P, TQ, 4], F32, tag="ssg_all")
            for i in range(TQ):
                ps = qkvp.tile([P, 3, P], F32, tag="qkv")
                for d in range(KC):
                    nc.tensor.matmul(
                        ps[:],
                        xT_all[:, d, ts(i, P)],
                        wT_all[:, d, hp::NHP, :],
                        start=(d == 0),
                        stop=(d == KC - 1),
                    )
                # stage q|k to bf16 SBUF (frees PSUM; ACT/DVE alternate)
                tu = scr.tile([P, 4, HD], BF16, tag=f"tu{i}", bufs=1)
                tu2 = tu[:].rearrange("p h d -> p (h d)")
                ps_qk = ps[:, 0:2, :].rearrange("p a c -> p (a c)")
                if i % 2 == 0:
                    nc.scalar.copy(tu2, ps_qk)
                else:
                    nc.vector.tensor_copy(tu2, ps_qk)
                tus.append(tu)
                # v: apply the input-RMSNorm factor, evict to bf16
                nc.vector.tensor_scalar_mul(
                    v_sb[i][:, 2 * hp : 2 * hp + 2, 0:HD],
                    ps[:, 2, :].rearrange("p (h d) -> p h d", d=HD),
                    rstd[:, i : i + 1],
                )
                # per-head sum of squares from the bf16 staging
                sqg = scr.tile([P, 4, HD], F32, tag="sqg")
                nc.gpsimd.tensor_mul(sqg[:], tu[:], tu[:])
                nc.vector.tensor_reduce(
                    ssg_all[:, i, :],
                    sqg[:],
                    axis=mybir.AxisListType.X,
                    op=mybir.AluOpType.add,
                )
            # batched norm factors: q cols get 1/sqrt(ssq+hd*eps) (folds the
            # 1/sqrt(hd) attention scale), k cols get 8x that
            lng = scr.tile([P, TQ, 4], F32, tag="lng")
            nc.scalar.activation(
                lng[:].rearrange("p i h -> p (i h)"),
                ssg_all[:].rearrange("p i h -> p (i h)"),
                AF.Ln,
                bias=eps64_t[:],
            )
            r_all = scr.tile([P, TQ, 4, 1], F32, tag="r_all")
            nc.scalar.activation(
                r_all[:, :, :, 0].rearrange("p i h -> p (i h)"),
                lng[:].rearrange("p i h -> p (i h)"),
                AF.Exp,
                scale=-0.5,
            )
            nc.gpsimd.tensor_scalar_mul(
                r_all[:, :, 2:4, 0], r_all[:, :, 2:4, 0], 8.0
            )
            for i in range(TQ):
                tn = scr.tile([P, 4, HD], BF16, tag="tn", bufs=3)
                nc.gpsimd.tensor_mul(
                    tn[:], tus[i][:], r_all[:, i, :, :].to_broadcast((P, 4, HD))
                )
                nc.sync.dma_start_transpose(
                    out=qkT[:, :, hp, ts(i, P)],
                    in_=tn[:].rearrange("p h d -> p (h d)"),
                )

        def emit_attention(hp):
            """scores -> exp -> AV -> normalize for head pair hp."""
            ets = {}
            for jt in range(TQ):
                for hh in range(2):
                    off = HD * hh
                    sc = scp.tile([P, L], F32, tag="sc")
                    for ic in range(2):
                        nc.tensor.matmul(
                            sc[:, ts(ic, 512)],
                            qkT[off : off + HD, 1, hp, ts(jt, P)],
                            qkT[off : off + HD, 0, hp, ts(ic, 512)],
                            start=True,
                            stop=True,
                            tile_position=(off, 0),
                        )
                    et = ets_pool.tile([P, L], BF16, tag=f"ets{jt}_{hh}")
                    nc.scalar.activation(et[:], sc[:], AF.Exp)
                    ets[(jt, hh)] = et

            rs = scr.tile([1, 2, L], F32, tag="rs", bufs=1)
            for hh in range(2):
                h = 2 * hp + hh
                for ic in range(2):
                    av = avp.tile([HD + 1, 512], F32, tag="av")
                    for jt in range(TQ):
                        nc.tensor.matmul(
                            av[:],
                            v_sb[jt][:, h, :],
                            ets[(jt, hh)][:, ts(ic, 512)],
                            start=(jt == 0),
                            stop=(jt == TQ - 1),
                        )
                    nc.vector.tensor_copy(
                        attnT[HD * hh : HD * hh + HD, hp, ts(ic, 512)],
                        av[0:HD, :],
                    )
                    if ic == 0:
                        nc.scalar.copy(rs[0:1, hh, ts(ic, 512)], av[HD : HD + 1, :])
                    else:
                        nc.vector.tensor_copy(
                            rs[0:1, hh, ts(ic, 512)], av[HD : HD + 1, :]
                        )
            rinv = scr.tile([1, 2, L], F32, tag="rinv", bufs=1)
            nc.vector.reciprocal_approx_fast(
                out=rinv[:].rearrange("o a l -> o (a l)"),
                in_=rs[:].rearrange("o a l -> o (a l)"),
            )
            rinv_b = scr.tile([1, 2, L], BF16, tag="rinv_b", bufs=1)
            nc.gpsimd.tensor_copy(rinv_b[:], rinv[:])
            for hh in range(2):
                off = HD * hh
                for ic in range(2):
                    bc = avp.tile([HD + 1, 512], F32, tag="av")
                    nc.tensor.matmul(
                        bc[0:HD, :],
                        ones_b[:],
                        rinv_b[0:1, hh, ts(ic, 512)],
                        start=True,
                        stop=True,
                    )
                    nc.vector.tensor_mul(
                        attnT[off : off + HD, hp, ts(ic, 512)],
                        attnT[off : off + HD, hp, ts(ic, 512)],
                        bc[0:HD, :],
                    )

        for hp in range(NHP + 1):
            if hp < NHP:
                emit_qkv(hp)
            if hp > 0:
                emit_attention(hp - 1)

        # ---- o_proj + residual ----
        for pair in range(TQ // 2):
            xs = scr.tile([P, 2, D], F32, tag="xs", bufs=1)
            nc.sync.dma_start(
                out=xs[:],
                in_=x_d[ts(pair, 2 * P), :].rearrange("(a p) d -> p a d", p=P),
            )
            osb = scr.tile([P, 2, D], F32, tag="osb", bufs=1)
            for j in range(2):
                i = 2 * pair + j
                o = scp.tile([P, L], F32, tag="sc")
                for c in range(KC):
                    nc.tensor.matmul(
                        o[:, 0:512],
                        attnT[:, c, ts(i, P)],
                        oT_all[:, c, 0:512],
                        start=(c == 0),
                        stop=(c == KC - 1),
                    )
                for c in range(KC):
                    nc.tensor.matmul(
                        o[:, 512:D],
                        attnT[:, c, ts(i, P)],
                        oT_all[:, c, 512:D],
                        start=(c == 0),
                        stop=(c == KC - 1),
                    )
                nc.vector.tensor_add(osb[:, j, 0:512], o[:, 0:512], xs[:, j, 0:512])
                nc.vector.tensor_add(osb[:, j, 512:D], o[:, 512:D], xs[:, j, 512:D])
            nc.sync.dma_start(
                out=out_d[ts(pair, 2 * P), :].rearrange("(a p) d -> p a d", p=P),
                in_=osb[:],
            )

    nc.compile()
    return nc


_NC = None


def _get_nc():
    global _NC
    if _NC is None:
        _NC = build_bass()
    return _NC


def make_in_maps(input_NHWD, qkv_weight, o_weight, o_scale):
    N = input_NHWD.shape[0]
    wT = np.ascontiguousarray(
        qkv_weight.reshape(3 * D, D).T.astype(np.float32)
    ).astype(ml_dtypes.bfloat16)
    oT = np.ascontiguousarray(
        (o_weight * o_scale[:, None]).T.astype(np.float32)
    ).astype(ml_dtypes.bfloat16)
    in_maps = []
    for i in range(N):
        xi = np.ascontiguousarray(input_NHWD[i].reshape(L, D).astype(np.float32))
        in_maps.append(
            {
                "x": xi,
                "xT": np.ascontiguousarray(xi.T).astype(ml_dtypes.bfloat16),
                "wT": wT,
                "oT": oT,
            }
        )
    return in_maps


def kernel(input_NHWD, qkv_weight, o_weight, o_scale):
    import time
    from concourse.bass_utils import run_bass_kernel_spmd

    input_NHWD = np.asarray(input_NHWD)
    N, H, W, _ = input_NHWD.shape
    nc = _get_nc()
    in_maps = make_in_maps(
        np.asarray(input_NHWD),
        np.asarray(qkv_weight),
        np.asarray(o_weight),
        np.asarray(o_scale),
    )
    last_err = None
    for attempt in range(3):
        try:
            res = run_bass_kernel_spmd(nc, in_maps, list(range(N)))
            out = np.stack([res.results[i]["out"] for i in range(N)], axis=0)
            return out.reshape(N, H, W, D).astype(np.float32)
        except Exception as e:  # transient device wedge: clear + retry
            last_err = e
            try:
                import jax

                jax.clear_caches()
                jax.clear_backends()
            except Exception:
                pass
            time.sleep(10)
    raise last_err


# revision 28
# speedup vs baseline: 1.6299x; 1.1246x over previous
"""Encoder self-attention (RMSNorm + fused QKV + qk-norm + SDPA + scaled o_proj
+ residual) on 8 NeuronCores, data-parallel over the batch dim N=8.

v3: head-pair-major software pipeline.
- QKV for one head pair via a step-sliced rhs (q|k|v 128-col blocks).
- qk-norm is scale-invariant => input-RMSNorm applied only to v.
- q/k PSUM evicted to bf16 staging; per-head sum-of-squares on Pool/DVE;
  norm factors batched per head pair as exp(-0.5*ln(ssq+hd*eps)) on ACT
  (one pinned activation table: ln/exp/square -> no table thrash).
- Transposed to [chan, tok] via one combined DMA-transpose per token tile.
- scores -> exp(bf16) -> AV with a ones column producing softmax
  denominators -> reciprocal_approx_fast -> bf16 K=1 outer-product
  broadcast matmul -> normalize.
- Emission is software-pipelined: QKV(hp+1) is emitted before attention(hp)
  so the PE never waits for fresh transposes.
"""

import numpy as np
import ml_dtypes
from contextlib import ExitStack

import concourse.bass as bass
import concourse.mybir as mybir
import concourse.tile as tile
from concourse import bacc
from concourse.bass import ts

F32 = mybir.dt.float32
BF16 = mybir.dt.bfloat16
AF = mybir.ActivationFunctionType

P = 128
D = 768
L = 1024
NH = 12
HD = 64
TQ = L // P      # 8 token tiles
KC = D // P      # 6 contraction chunks
NHP = NH // 2    # 6 head pairs
EPS = 1e-6
# act_info.json act_func_sets index of natural_log_exp_and_others
ACT_SET_LN_EXP = 6


def build_bass():
    nc = bacc.Bacc(None, target_bir_lowering=False)

    x_d = nc.dram_tensor("x", [L, D], F32, kind="ExternalInput")
    xT_d = nc.dram_tensor("xT", [D, L], BF16, kind="ExternalInput")
    wT_d = nc.dram_tensor("wT", [D, 3 * D], BF16, kind="ExternalInput")
    oT_d = nc.dram_tensor("oT", [D, D], BF16, kind="ExternalInput")
    out_d = nc.dram_tensor("out", [L, D], F32, kind="ExternalOutput")

    with tile.TileContext(nc) as tc, ExitStack() as ctx:
        persist = ctx.enter_context(tc.tile_pool(name="persist", bufs=1))
        ets_pool = ctx.enter_context(tc.tile_pool(name="ets", bufs=2))
        scr = ctx.enter_context(tc.tile_pool(name="scr", bufs=2))
        qkvp = ctx.enter_context(tc.tile_pool(name="qkvp", bufs=2, space="PSUM"))
        scp = ctx.enter_context(tc.tile_pool(name="scp", bufs=2, space="PSUM"))
        avp = ctx.enter_context(tc.tile_pool(name="avp", bufs=2, space="PSUM"))

        # pin the ln/exp/square activation table once: every ACT func used
        # here is served by it, so no per-activation table reloads
        atl = mybir.InstLoadActFuncSet(
            name=nc.get_next_instruction_name(),
            ins=[],
            outs=[],
            act_func_set_id=ACT_SET_LN_EXP,
        )
        nc.scalar.add_instruction(atl)

        # ---- persistent SBUF ----
        xT_all = persist.tile([P, KC, L], BF16, tag="xT_all", name="xT_all")
        wT_all = persist.tile([P, KC, 18, P], BF16, tag="wT_all", name="wT_all")
        # qkT[:, 0] = qnT, qkT[:, 1] = knT  (chan-major, [chan, tok] blocks)
        qkT = persist.tile([P, 2, NHP, L], BF16, tag="qkT", name="qkT")
        v_sb = [
            persist.tile([P, NH, HD + 1], BF16, tag=f"v{j}", name=f"v{j}")
            for j in range(TQ)
        ]
        attnT = persist.tile([P, KC, L], BF16, tag="attnT", name="attnT")
        oT_all = persist.tile([P, KC, D], BF16, tag="oT_all", name="oT_all")
        rstd = persist.tile([P, TQ], F32, tag="rstd")
        ones_b = persist.tile([1, HD], BF16, tag="ones_b")
        nc.gpsimd.memset(ones_b[:], 1.0)
        eps_t = persist.tile([P, 1], F32, tag="eps_t")
        nc.gpsimd.memset(eps_t[:], EPS)
        eps64_t = persist.tile([P, 1], F32, tag="eps64_t")
        nc.gpsimd.memset(eps64_t[:], HD * EPS)

        # ---- input DMAs ----
        for d in range(KC):
            nc.sync.dma_start(out=xT_all[:, d, :], in_=xT_d[ts(d, P), :])
            nc.sync.dma_start(
                out=wT_all[:, d, :, :],
                in_=wT_d[ts(d, P), :].rearrange("p (g c) -> p g c", c=P),
            )
        nc.scalar.dma_start(
            out=oT_all[:], in_=oT_d[:].rearrange("(c p) e -> p c e", p=P)
        )
        for i in range(TQ):
            nc.gpsimd.memset(v_sb[i][:, :, HD : HD + 1], 1.0)

        # ---- rstd, pipelined per token tile so hp=0 evictions start early ----
        ssq_all = scr.tile([P, TQ], F32, tag="ssq_all", bufs=1)
        lnr = scr.tile([P, TQ], F32, tag="lnr", bufs=1)
        for pair in range(TQ // 2):
            xs = scr.tile([P, 2, D], F32, tag="xs", bufs=1)
            nc.scalar.dma_start(
                out=xs[:],
                in_=x_d[ts(pair, 2 * P), :].rearrange("(a p) d -> p a d", p=P),
            )
            sqs = scr.tile([P, D], F32, tag="sqs", bufs=1)
            for j in range(2):
                i = 2 * pair + j
                nc.scalar.activation(
                    sqs[:], xs[:, j, :], AF.Square,
                    accum_out=ssq_all[:, i : i + 1],
                )
                nc.scalar.activation(
                    lnr[:, i : i + 1], ssq_all[:, i : i + 1], AF.Ln,
                    scale=1.0 / D, bias=eps_t[:],
                )
                nc.scalar.activation(
                    rstd[:, i : i + 1], lnr[:, i : i + 1], AF.Exp, scale=-0.5
                )

        # ================= software-pipelined main loop =================
        def emit_qkv(hp):
            """QKV + staging + sum-of-squares + batched norm + transposes."""
            tus = []
            ssg_all = scr.tile([P, TQ, 4], F32, tag="ssg_all")
            for i in range(TQ):
                ps = qkvp.tile([P, 3, P], F32, tag="qkv")
                for d in range(KC):
                    nc.tensor.matmul(
                        ps[:],
                        xT_all[:, d, ts(i, P)],
                        wT_all[:, d, hp::NHP, :],
                        start=(d == 0),
                        stop=(d == KC - 1),
                    )
                # stage q|k to bf16 SBUF (frees PSUM; ACT/DVE alternate)
                tu = scr.tile([P, 4, HD], BF16, tag=f"tu{i}", bufs=1)
                tu2 = tu[:].rearrange("p h d -> p (h d)")
                ps_qk = ps[:, 0:2, :].rearrange("p a c -> p (a c)")
                if i % 2 == 0:
                    nc.vector.tensor_copy(tu2, ps_qk)
                else:
                    nc.scalar.copy(tu2, ps_qk)
                tus.append(tu)
                # v: apply the input-RMSNorm factor, evict to bf16
                nc.vector.tensor_scalar_mul(
                    v_sb[i][:, 2 * hp : 2 * hp + 2, 0:HD],
                    ps[:, 2, :].rearrange("p (h d) -> p h d", d=HD),
                    rstd[:, i : i + 1],
                )
                # per-head sum of squares from the bf16 staging
                sqg = scr.tile([P, 4, HD], F32, tag="sqg")
                if i % 2 == 0:
                    nc.vector.tensor_mul(sqg[:], tu[:], tu[:])
                else:
                    nc.scalar.activation(
                        sqg[:].rearrange("p h d -> p (h d)"),
                        tu[:].rearrange("p h d -> p (h d)"),
                        AF.Square,
                    )
                nc.vector.tensor_reduce(
                    ssg_all[:, i, :],
                    sqg[:],
                    axis=mybir.AxisListType.X,
                    op=mybir.AluOpType.add,
                )
            # batched norm factors: q cols get 1/sqrt(ssq+hd*eps) (folds the
            # 1/sqrt(hd) attention scale), k cols get 8x that
            lng = scr.tile([P, TQ, 4], F32, tag="lng")
            nc.scalar.activation(
                lng[:].rearrange("p i h -> p (i h)"),
                ssg_all[:].rearrange("p i h -> p (i h)"),
                AF.Ln,
                bias=eps64_t[:],
            )
            r_all = scr.tile([P, TQ, 4, 1], F32, tag="r_all")
            nc.scalar.activation(
                r_all[:, :, :, 0].rearrange("p i h -> p (i h)"),
                lng[:].rearrange("p i h -> p (i h)"),
                AF.Exp,
                scale=-0.5,
            )
            nc.vector.tensor_scalar_mul(
                r_all[:, :, 2:4, 0], r_all[:, :, 2:4, 0], 8.0
            )
            for i in range(TQ):
                tn = scr.tile([P, 4, HD], BF16, tag="tn", bufs=3)
                nc.gpsimd.tensor_mul(
                    tn[:], tus[i][:], r_all[:, i, :, :].to_broadcast((P, 4, HD))
                )
                nc.sync.dma_start_transpose(
                    out=qkT[:, :, hp, ts(i, P)],
                    in_=tn[:].rearrange("p h d -> p (h d)"),
                )

        def emit_attention(hp):
            """scores -> exp -> AV -> normalize for head pair hp."""
            ets = {}
            rs = scr.tile([1, 2, L], F32, tag="rs", bufs=1)
            for hh in range(2):
                off = HD * hh
                h = 2 * hp + hh
                for jt in range(TQ):
                    sc = scp.tile([P, L], F32, tag="sc")
                    for ic in range(2):
                        nc.tensor.matmul(
                            sc[:, ts(ic, 512)],
                            qkT[off : off + HD, 1, hp, ts(jt, P)],
                            qkT[off : off + HD, 0, hp, ts(ic, 512)],
                            start=True,
                            stop=True,
                            tile_position=(off, 0),
                        )
                    et = ets_pool.tile([P, L], BF16, tag=f"ets{jt}_{hh}")
                    nc.scalar.activation(et[:], sc[:], AF.Exp)
                    ets[(jt, hh)] = et
                for ic in range(2):
                    av = avp.tile([HD + 1, 512], F32, tag="av")
                    for jt in range(TQ):
                        nc.tensor.matmul(
                            av[:],
                            v_sb[jt][:, h, :],
                            ets[(jt, hh)][:, ts(ic, 512)],
                            start=(jt == 0),
                            stop=(jt == TQ - 1),
                        )
                    nc.vector.tensor_copy(
                        attnT[HD * hh : HD * hh + HD, hp, ts(ic, 512)],
                        av[0:HD, :],
                    )
                    nc.vector.tensor_copy(
                        rs[0:1, hh, ts(ic, 512)], av[HD : HD + 1, :]
                    )
            rinv = scr.tile([1, 2, L], F32, tag="rinv", bufs=1)
            nc.vector.reciprocal_approx_fast(
                out=rinv[:].rearrange("o a l -> o (a l)"),
                in_=rs[:].rearrange("o a l -> o (a l)"),
            )
            rinv_b = scr.tile([1, 2, L], BF16, tag="rinv_b", bufs=1)
            nc.gpsimd.tensor_copy(rinv_b[:], rinv[:])
            for hh in range(2):
                off = HD * hh
                for ic in range(2):
                    bc = avp.tile([HD + 1, 512], F32, tag="av")
                    nc.tensor.matmul(
                        bc[0:HD, :],
                        ones_b[:],
                        rinv_b[0:1, hh, ts(ic, 512)],
                        start=True,
                        stop=True,
                    )
                    nc.vector.tensor_mul(
                        attnT[off : off + HD, hp, ts(ic, 512)],
                        attnT[off : off + HD, hp, ts(ic, 512)],
                        bc[0:HD, :],
                    )

        for hp in range(NHP + 1):
            if hp < NHP:
                emit_qkv(hp)
            if hp > 0:
                emit_attention(hp - 1)

        # ---- o_proj + residual ----
        for pair in range(TQ // 2):
            xs = scr.tile([P, 2, D], F32, tag="xs", bufs=1)
            nc.sync.dma_start(
                out=xs[:],
                in_=x_d[ts(pair, 2 * P), :].rearrange("(a p) d -> p a d", p=P),
            )
            osb = scr.tile([P, 2, D], F32, tag="osb", bufs=1)
            for j in range(2):
                i = 2 * pair + j
                o = scp.tile([P, L], F32, tag="sc")
                for c in range(KC):
                    nc.tensor.matmul(
                        o[:, 0:512],
                        attnT[:, c, ts(i, P)],
                        oT_all[:, c, 0:512],
                        start=(c == 0),
                        stop=(c == KC - 1),
                    )
                for c in range(KC):
                    nc.tensor.matmul(
                        o[:, 512:D],
                        attnT[:, c, ts(i, P)],
                        oT_all[:, c, 512:D],
                        start=(c == 0),
                        stop=(c == KC - 1),
                    )
                nc.vector.tensor_add(osb[:, j, 0:512], o[:, 0:512], xs[:, j, 0:512])
                nc.vector.tensor_add(osb[:, j, 512:D], o[:, 512:D], xs[:, j, 512:D])
            nc.sync.dma_start(
                out=out_d[ts(pair, 2 * P), :].rearrange("(a p) d -> p a d", p=P),
                in_=osb[:],
            )

    nc.compile()
    return nc


_NC = None


def _get_nc():
    global _NC
    if _NC is None:
        _NC = build_bass()
    return _NC


def make_in_maps(input_NHWD, qkv_weight, o_weight, o_scale):
    N = input_NHWD.shape[0]
    wT = np.ascontiguousarray(
        qkv_weight.reshape(3 * D, D).T.astype(np.float32)
    ).astype(ml_dtypes.bfloat16)
    oT = np.ascontiguousarray(
        (o_weight * o_scale[:, None]).T.astype(np.float32)
    ).astype(ml_dtypes.bfloat16)
    in_maps = []
    for i in range(N):
        xi = np.ascontiguousarray(input_NHWD[i].reshape(L, D).astype(np.float32))
        in_maps.append(
            {
                "x": xi,
                "xT": np.ascontiguousarray(xi.T).astype(ml_dtypes.bfloat16),
                "wT": wT,
                "oT": oT,
            }
        )
    return in_maps


def kernel(input_NHWD, qkv_weight, o_weight, o_scale):
    import time
    from concourse.bass_utils import run_bass_kernel_spmd

    input_NHWD = np.asarray(input_NHWD)
    N, H, W, _ = input_NHWD.shape
    nc = _get_nc()
    in_maps = make_in_maps(
        np.asarray(input_NHWD),
        np.asarray(qkv_weight),
        np.asarray(o_weight),
        np.asarray(o_scale),
    )
    last_err = None
    for attempt in range(3):
        try:
            res = run_bass_kernel_spmd(nc, in_maps, list(range(N)))
            out = np.stack([res.results[i]["out"] for i in range(N)], axis=0)
            return out.reshape(N, H, W, D).astype(np.float32)
        except Exception as e:  # transient device wedge: clear + retry
            last_err = e
            try:
                import jax

                jax.clear_caches()
                jax.clear_backends()
            except Exception:
                pass
            time.sleep(10)
    raise last_err


# revision 31
# speedup vs baseline: 1.8379x; 1.1276x over previous
"""Encoder self-attention (RMSNorm + fused QKV + qk-norm + SDPA + scaled o_proj
+ residual) on 8 NeuronCores, data-parallel over the batch dim N=8.

v3: head-pair-major software pipeline.
- QKV for one head pair via a step-sliced rhs (q|k|v 128-col blocks).
- qk-norm is scale-invariant => input-RMSNorm applied only to v.
- q/k PSUM evicted to bf16 staging; per-head sum-of-squares on Pool/DVE;
  norm factors batched per head pair as exp(-0.5*ln(ssq+hd*eps)) on ACT
  (one pinned activation table: ln/exp/square -> no table thrash).
- Transposed to [chan, tok] via one combined DMA-transpose per token tile.
- scores -> exp(bf16) -> AV with a ones column producing softmax
  denominators -> reciprocal_approx_fast -> bf16 K=1 outer-product
  broadcast matmul -> normalize.
- Emission is software-pipelined: QKV(hp+1) is emitted before attention(hp)
  so the PE never waits for fresh transposes.
"""

import numpy as np
import ml_dtypes
from contextlib import ExitStack

import concourse.bass as bass
import concourse.mybir as mybir
import concourse.tile as tile
from concourse import bacc
from concourse.bass import ts

F32 = mybir.dt.float32
BF16 = mybir.dt.bfloat16
AF = mybir.ActivationFunctionType

P = 128
D = 768
L = 1024
NH = 12
HD = 64
TQ = L // P      # 8 token tiles
KC = D // P      # 6 contraction chunks
NHP = NH // 2    # 6 head pairs
EPS = 1e-6
# act_info.json act_func_sets index of natural_log_exp_and_others
ACT_SET_LN_EXP = 6


def build_bass():
    nc = bacc.Bacc(None, target_bir_lowering=False)

    x_d = nc.dram_tensor("x", [L, D], F32, kind="ExternalInput")
    xT_d = nc.dram_tensor("xT", [D, L], BF16, kind="ExternalInput")
    wT_d = nc.dram_tensor("wT", [D, 3 * D], BF16, kind="ExternalInput")
    oT_d = nc.dram_tensor("oT", [D, D], BF16, kind="ExternalInput")
    out_d = nc.dram_tensor("out", [L, D], F32, kind="ExternalOutput")

    with tile.TileContext(nc) as tc, ExitStack() as ctx:
        persist = ctx.enter_context(tc.tile_pool(name="persist", bufs=1))
        ets_pool = ctx.enter_context(tc.tile_pool(name="ets", bufs=2))
        scr = ctx.enter_context(tc.tile_pool(name="scr", bufs=2))
        qkvp = ctx.enter_context(tc.tile_pool(name="qkvp", bufs=2, space="PSUM"))
        scp = ctx.enter_context(tc.tile_pool(name="scp", bufs=2, space="PSUM"))
        avp = ctx.enter_context(tc.tile_pool(name="avp", bufs=2, space="PSUM"))

        # pin the ln/exp/square activation table once: every ACT func used
        # here is served by it, so no per-activation table reloads
        atl = mybir.InstLoadActFuncSet(
            name=nc.get_next_instruction_name(),
            ins=[],
            outs=[],
            act_func_set_id=ACT_SET_LN_EXP,
        )
        nc.scalar.add_instruction(atl)

        # ---- persistent SBUF ----
        xT_all = persist.tile([P, KC, L], BF16, tag="xT_all", name="xT_all")
        wT_all = persist.tile([P, KC, 18, P], BF16, tag="wT_all", name="wT_all")
        # qkT[:, 0] = qnT, qkT[:, 1] = knT  (chan-major, [chan, tok] blocks)
        qkT = persist.tile([P, 2, NHP, L], BF16, tag="qkT", name="qkT")
        v_sb = [
            persist.tile([P, NH, HD + 1], BF16, tag=f"v{j}", name=f"v{j}")
            for j in range(TQ)
        ]
        attnT = persist.tile([P, KC, L], BF16, tag="attnT", name="attnT")
        oT_all = persist.tile([P, KC, D], BF16, tag="oT_all", name="oT_all")
        rstd = persist.tile([P, TQ], F32, tag="rstd")
        ones_b = persist.tile([1, HD], BF16, tag="ones_b")
        nc.gpsimd.memset(ones_b[:], 1.0)
        eps_t = persist.tile([P, 1], F32, tag="eps_t")
        nc.gpsimd.memset(eps_t[:], EPS)
        eps64_t = persist.tile([P, 1], F32, tag="eps64_t")
        nc.gpsimd.memset(eps64_t[:], HD * EPS)

        # ---- input DMAs ----
        for d in range(KC):
            nc.sync.dma_start(out=xT_all[:, d, :], in_=xT_d[ts(d, P), :])
            nc.sync.dma_start(
                out=wT_all[:, d, :, :],
                in_=wT_d[ts(d, P), :].rearrange("p (g c) -> p g c", c=P),
            )
        nc.scalar.dma_start(
            out=oT_all[:], in_=oT_d[:].rearrange("(c p) e -> p c e", p=P)
        )
        for i in range(TQ):
            nc.gpsimd.memset(v_sb[i][:, :, HD : HD + 1], 1.0)

        # ---- rstd, pipelined per token tile so hp=0 evictions start early ----
        ssq_all = scr.tile([P, TQ], F32, tag="ssq_all", bufs=1)
        lnr = scr.tile([P, TQ], F32, tag="lnr", bufs=1)
        for pair in range(TQ // 2):
            xs = scr.tile([P, 2, D], F32, tag="xs", bufs=1)
            nc.scalar.dma_start(
                out=xs[:],
                in_=x_d[ts(pair, 2 * P), :].rearrange("(a p) d -> p a d", p=P),
            )
            sqs = scr.tile([P, D], F32, tag="sqs", bufs=1)
            for j in range(2):
                i = 2 * pair + j
                nc.scalar.activation(
                    sqs[:], xs[:, j, :], AF.Square,
                    accum_out=ssq_all[:, i : i + 1],
                )
                nc.scalar.activation(
                    lnr[:, i : i + 1], ssq_all[:, i : i + 1], AF.Ln,
                    scale=1.0 / D, bias=eps_t[:],
                )
                nc.scalar.activation(
                    rstd[:, i : i + 1], lnr[:, i : i + 1], AF.Exp, scale=-0.5
                )

        # ================= software-pipelined main loop =================
        def emit_qkv(hp):
            """QKV + staging + sum-of-squares + batched norm + transposes."""
            tus = []
            ssg_all = scr.tile([P, TQ, 4], F32, tag="ssg_all")
            for i in range(TQ):
                ps = qkvp.tile([P, 3, P], F32, tag="qkv")
                for d in range(KC):
                    nc.tensor.matmul(
                        ps[:],
                        xT_all[:, d, ts(i, P)],
                        wT_all[:, d, hp::NHP, :],
                        start=(d == 0),
                        stop=(d == KC - 1),
                    )
                # stage q|k to bf16 SBUF (frees PSUM; ACT/DVE alternate)
                tu = scr.tile([P, 4, HD], BF16, tag=f"tu{i}", bufs=1)
                tu2 = tu[:].rearrange("p h d -> p (h d)")
                ps_qk = ps[:, 0:2, :].rearrange("p a c -> p (a c)")
                nc.vector.tensor_copy(tu2, ps_qk)
                tus.append(tu)
                # v: apply the input-RMSNorm factor, evict to bf16
                nc.vector.tensor_scalar_mul(
                    v_sb[i][:, 2 * hp : 2 * hp + 2, 0:HD],
                    ps[:, 2, :].rearrange("p (h d) -> p h d", d=HD),
                    rstd[:, i : i + 1],
                )
                # per-head sum of squares from the bf16 staging
                sqg = scr.tile([P, 4, HD], BF16, tag="sqg")
                nc.vector.tensor_mul(sqg[:], tu[:], tu[:])
                nc.vector.tensor_reduce(
                    ssg_all[:, i, :],
                    sqg[:],
                    axis=mybir.AxisListType.X,
                    op=mybir.AluOpType.add,
                )
            # batched norm factors: q cols get 1/sqrt(ssq+hd*eps) (folds the
            # 1/sqrt(hd) attention scale), k cols get 8x that
            lng = scr.tile([P, TQ, 4], F32, tag="lng")
            nc.scalar.activation(
                lng[:].rearrange("p i h -> p (i h)"),
                ssg_all[:].rearrange("p i h -> p (i h)"),
                AF.Ln,
                bias=eps64_t[:],
            )
            r_all = scr.tile([P, TQ, 4, 1], F32, tag="r_all")
            nc.scalar.activation(
                r_all[:, :, :, 0].rearrange("p i h -> p (i h)"),
                lng[:].rearrange("p i h -> p (i h)"),
                AF.Exp,
                scale=-0.5,
            )
            nc.vector.tensor_scalar_mul(
                r_all[:, :, 2:4, 0], r_all[:, :, 2:4, 0], 8.0
            )
            for i in range(TQ):
                tn = scr.tile([P, 4, HD], BF16, tag="tn", bufs=3)
                nc.gpsimd.tensor_mul(
                    tn[:], tus[i][:], r_all[:, i, :, :].to_broadcast((P, 4, HD))
                )
                nc.sync.dma_start_transpose(
                    out=qkT[:, :, hp, ts(i, P)],
                    in_=tn[:].rearrange("p h d -> p (h d)"),
                )

        def emit_attention(hp):
            """scores -> exp -> AV -> normalize for head pair hp."""
            ets = {}
            rinv_b = scr.tile([1, 2, L], BF16, tag="rinv_b", bufs=1)
            for hh in range(2):
                off = HD * hh
                h = 2 * hp + hh
                for jt in range(TQ):
                    sc = scp.tile([P, L], F32, tag="sc")
                    for ic in range(2):
                        nc.tensor.matmul(
                            sc[:, ts(ic, 512)],
                            qkT[off : off + HD, 1, hp, ts(jt, P)],
                            qkT[off : off + HD, 0, hp, ts(ic, 512)],
                            start=True,
                            stop=True,
                            tile_position=(off, 0),
                        )
                    et = ets_pool.tile([P, L], BF16, tag=f"ets{jt}_{hh}")
                    nc.scalar.activation(et[:], sc[:], AF.Exp)
                    ets[(jt, hh)] = et
                for ic in range(2):
                    av = avp.tile([HD + 1, 512], F32, tag="av")
                    for jt in range(TQ):
                        nc.tensor.matmul(
                            av[:],
                            v_sb[jt][:, h, :],
                            ets[(jt, hh)][:, ts(ic, 512)],
                            start=(jt == 0),
                            stop=(jt == TQ - 1),
                        )
                    nc.vector.tensor_copy(
                        attnT[HD * hh : HD * hh + HD, hp, ts(ic, 512)],
                        av[0:HD, :],
                    )
                    rl = scr.tile([1, 512], F32, tag="rl")
                    rl2 = scr.tile([1, 512], F32, tag="rl2")
                    if hh == 0:
                        nc.scalar.copy(rl[:], av[HD : HD + 1, :])
                    else:
                        nc.vector.tensor_copy(rl[:], av[HD : HD + 1, :])
                    nc.vector.reciprocal_approx_fast(out=rl2[:], in_=rl[:])
                    if hh == 0:
                        nc.vector.tensor_copy(rinv_b[0:1, hh, ts(ic, 512)], rl2[:])
                    else:
                        nc.scalar.copy(rinv_b[0:1, hh, ts(ic, 512)], rl2[:])
            for hh in range(2):
                off = HD * hh
                for ic in range(2):
                    bc = avp.tile([HD + 1, 512], F32, tag="av")
                    nc.tensor.matmul(
                        bc[0:HD, :],
                        ones_b[:],
                        rinv_b[0:1, hh, ts(ic, 512)],
                        start=True,
                        stop=True,
                    )
                    nc.vector.tensor_mul(
                        attnT[off : off + HD, hp, ts(ic, 512)],
                        attnT[off : off + HD, hp, ts(ic, 512)],
                        bc[0:HD, :],
                    )

        for hp in range(NHP + 1):
            if hp < NHP:
                emit_qkv(hp)
            if hp > 0:
                emit_attention(hp - 1)

        # ---- o_proj + residual ----
        for pair in range(TQ // 2):
            xs = scr.tile([P, 2, D], F32, tag="xs", bufs=1)
            nc.sync.dma_start(
                out=xs[:],
                in_=x_d[ts(pair, 2 * P), :].rearrange("(a p) d -> p a d", p=P),
            )
            osb = scr.tile([P, 2, D], F32, tag="osb", bufs=1)
            for j in range(2):
                i = 2 * pair + j
                o = scp.tile([P, L], F32, tag="sc")
                for c in range(KC):
                    nc.tensor.matmul(
                        o[:, 0:512],
                        attnT[:, c, ts(i, P)],
                        oT_all[:, c, 0:512],
                        start=(c == 0),
                        stop=(c == KC - 1),
                    )
                for c in range(KC):
                    nc.tensor.matmul(
                        o[:, 512:D],
                        attnT[:, c, ts(i, P)],
                        oT_all[:, c, 512:D],
                        start=(c == 0),
                        stop=(c == KC - 1),
                    )
                nc.vector.tensor_add(osb[:, j, 0:512], o[:, 0:512], xs[:, j, 0:512])
                nc.vector.tensor_add(osb[:, j, 512:D], o[:, 512:D], xs[:, j, 512:D])
            nc.sync.dma_start(
                out=out_d[ts(pair, 2 * P), :].rearrange("(a p) d -> p a d", p=P),
                in_=osb[:],
            )

    nc.compile()
    return nc


_NC = None


def _get_nc():
    global _NC
    if _NC is None:
        _NC = build_bass()
    return _NC


def make_in_maps(input_NHWD, qkv_weight, o_weight, o_scale):
    N = input_NHWD.shape[0]
    wT = np.ascontiguousarray(
        qkv_weight.reshape(3 * D, D).T.astype(np.float32)
    ).astype(ml_dtypes.bfloat16)
    oT = np.ascontiguousarray(
        (o_weight * o_scale[:, None]).T.astype(np.float32)
    ).astype(ml_dtypes.bfloat16)
    in_maps = []
    for i in range(N):
        xi = np.ascontiguousarray(input_NHWD[i].reshape(L, D).astype(np.float32))
        in_maps.append(
            {
                "x": xi,
                "xT": np.ascontiguousarray(xi.T).astype(ml_dtypes.bfloat16),
                "wT": wT,
                "oT": oT,
            }
        )
    return in_maps


def kernel(input_NHWD, qkv_weight, o_weight, o_scale):
    import time
    from concourse.bass_utils import run_bass_kernel_spmd

    input_NHWD = np.asarray(input_NHWD)
    N, H, W, _ = input_NHWD.shape
    nc = _get_nc()
    in_maps = make_in_maps(
        np.asarray(input_NHWD),
        np.asarray(qkv_weight),
        np.asarray(o_weight),
        np.asarray(o_scale),
    )
    last_err = None
    for attempt in range(3):
        try:
            res = run_bass_kernel_spmd(nc, in_maps, list(range(N)))
            out = np.stack([res.results[i]["out"] for i in range(N)], axis=0)
            return out.reshape(N, H, W, D).astype(np.float32)
        except Exception as e:  # transient device wedge: clear + retry
            last_err = e
            try:
                import jax

                jax.clear_caches()
                jax.clear_backends()
            except Exception:
                pass
            time.sleep(10)
    raise last_err
